# revision 7
# baseline (speedup 1.0000x reference)
"""DeterminantHead (FermiNet-style two-determinant head) on 8 Trainium2 cores.

Strategy: pure data-parallel over the walker batch (512 walkers/core).
Per core:
  - layer1  h^T[e, r] = gelu(W1^T f^T + b1)   (fp32r matmuls, K=256 in 2 chunks)
  - layer2  orb^T[k, r] = W2'^T h^T + b2'     (W2' = W2 @ Q, Q fixed orthogonal:
            right-preconditions the orbital matrices so no-pivot Gaussian
            elimination is stable; det(Q) = +-1 leaves log|det| unchanged)
  - strided PE transposes -> per-walker matrices A[b, spin, n, k] in SBUF
  - batched in-place Gaussian elimination on the DVE; log|det| = sum log|pivot|
  - envelope handled analytically: det(diag(env) M) = prod(env) det(M), so the
    kernel adds sum_n log(env[b, n]) instead of scaling the matrix.
Inputs are repacked host-side (transpose of features, coordinate split) so the
device only does well-shaped DMAs; the orthogonal mix Q is folded into W2/b2.
"""
import numpy as np

import concourse.bacc as bacc
import concourse.mybir as mybir
from concourse import tile
from concourse.bass_utils import run_bass_kernel_spmd
from concourse.masks import make_identity

AF = mybir.ActivationFunctionType
OP = mybir.AluOpType
F32 = mybir.dt.float32
F32R = mybir.dt.float32r

N_CORES = 8
B = 4096
N = 32          # electrons per spin
D = 256
A_ATOM = 16
BL = B // N_CORES       # walkers per core
SUPW = 128              # walkers per super-block
BLKW = 16               # walkers per matmul chunk
RCH = BLKW * N          # 512 rows per matmul chunk
NSUP = BL // SUPW


def mix_q():
    rngq = np.random.default_rng(12345)
    return np.linalg.qr(rngq.standard_normal((32, 32)))[0].astype(np.float32)


def declare(nc):
    t = {}
    t["fT"] = nc.dram_tensor("fT", [2, 2, 128, BL * N], F32, kind="ExternalInput")
    t["cxyz"] = nc.dram_tensor("cxyz", [3, 128, NSUP * 64], F32, kind="ExternalInput")
    t["W1p"] = nc.dram_tensor("W1p", [2, 2, 2, 128, 128], F32, kind="ExternalInput")
    t["b1p"] = nc.dram_tensor("b1p", [2, 2, 128, 1], F32, kind="ExternalInput")
    t["W2p"] = nc.dram_tensor("W2p", [2, 2, 128, 32], F32, kind="ExternalInput")
    t["b2c"] = nc.dram_tensor("b2c", [64, 1], F32, kind="ExternalInput")
    t["axyz"] = nc.dram_tensor("axyz", [3, 128, A_ATOM], F32, kind="ExternalInput")
    t["sigc"] = nc.dram_tensor("sigc", [2, 128, A_ATOM], F32, kind="ExternalInput")
    t["pic"] = nc.dram_tensor("pic", [2, 128, A_ATOM], F32, kind="ExternalInput")
    t["out"] = nc.dram_tensor("out", [128, NSUP], F32, kind="ExternalOutput")
    return t


def eliminate(nc, sb, A4, g, eng):
    P = 128
    fac = sb.tile([P, g * N], F32, tag="fac", name="fac")
    fv = fac[:].rearrange("p (s i) -> p s i", s=g)
    tmp = sb.tile([P, g * N * N], F32, tag="elimtmp", name="elimtmp")
    t4 = tmp[:].rearrange("p (s n k) -> p s n k", s=g, n=N)
    rcp = sb.tile([P, g], F32, tag="rcp", name="rcp")
    ab2 = sb.tile([P, g * 2], F32, tag="ab2", name="ab2")
    ab2v = ab2[:].rearrange("p (s i) -> p s i", s=g)
    msk = sb.tile([P, g], F32, tag="msk", name="msk")
    dswp = sb.tile([P, g * N], F32, tag="dswp", name="dswp")
    dsv = dswp[:].rearrange("p (s k) -> p s k", s=g)
    for j in range(N - 1):
        m = N - 1 - j
        w = N - j
        # window-2 pivoting: virtually swap rows j/j+1 when |A[j+1,j]| > |A[j,j]|
        col2 = A4[:, :, j:j + 2, j]
        eng.tensor_tensor(ab2v, col2, col2, op=OP.mult)
        eng.tensor_tensor(msk[:], ab2v[:, :, 1], ab2v[:, :, 0], op=OP.is_gt)
        dk = dsv[:, :, :w]
        eng.tensor_tensor(dk, A4[:, :, j, j:], A4[:, :, j + 1, j:], op=OP.subtract)
        eng.tensor_tensor(dk, dk, msk[:].unsqueeze(2).broadcast_to([P, g, w]), op=OP.mult)
        eng.tensor_tensor(A4[:, :, j, j:], A4[:, :, j, j:], dk, op=OP.subtract)
        eng.tensor_tensor(A4[:, :, j + 1, j:], A4[:, :, j + 1, j:], dk, op=OP.add)
        nc.vector.reciprocal(rcp[:], A4[:, :, j, j])
        facv = fv[:, :, :m]
        eng.tensor_tensor(facv, A4[:, :, j + 1:, j],
                          rcp[:].unsqueeze(2).broadcast_to([P, g, m]), op=OP.mult)
        eng.tensor_tensor(t4[:, :, j + 1:, j + 1:],
                          facv.unsqueeze(3).broadcast_to([P, g, m, m]),
                          A4[:, :, j, j + 1:].unsqueeze(2).broadcast_to([P, g, m, m]),
                          op=OP.mult)
        eng.tensor_tensor(A4[:, :, j + 1:, j + 1:], A4[:, :, j + 1:, j + 1:],
                          t4[:, :, j + 1:, j + 1:], op=OP.subtract)


def build(nc):
    nblk = SUPW // BLKW
    t = declare(nc)
    with tile.TileContext(nc) as tc:
        with tc.tile_pool(name="cst", bufs=1) as cst, \
             tc.tile_pool(name="ftp", bufs=3) as ftp, \
             tc.tile_pool(name="hp", bufs=2) as hp, \
             tc.tile_pool(name="orbp", bufs=1) as orbp, \
             tc.tile_pool(name="slv", bufs=2) as slv, \
             tc.tile_pool(name="sb", bufs=1) as sb, \
             tc.tile_pool(name="envp", bufs=2) as envp, \
             tc.tile_pool(name="psH", bufs=1, space="PSUM") as psH, \
             tc.tile_pool(name="psO", bufs=1, space="PSUM") as psO, \
             tc.tile_pool(name="psT", bufs=2, space="PSUM") as psT:
            ident = cst.tile([64, 64], F32)
            make_identity(nc, ident[:])
            w1t, w2t, b1t = {}, {}, {}
            for s in range(2):
                for kc in range(2):
                    for ec in range(2):
                        w = cst.tile([128, 128], F32, tag=f"w1_{s}{kc}{ec}", name=f"w1_{s}{kc}{ec}")
                        nc.sync.dma_start(w[:], t["W1p"][s, kc, ec])
                        w1t[s, kc, ec] = w
                for ec in range(2):
                    w = cst.tile([128, 32], F32, tag=f"w2_{s}{ec}", name=f"w2_{s}{ec}")
                    nc.sync.dma_start(w[:], t["W2p"][s, ec])
                    w2t[s, ec] = w
                    b = cst.tile([128, 1], F32, tag=f"b1_{s}{ec}", name=f"b1_{s}{ec}")
                    nc.sync.dma_start(b[:], t["b1p"][s, ec])
                    b1t[s, ec] = b
            b2u = cst.tile([32, 1], F32)
            nc.sync.dma_start(b2u[:], t["b2c"][0:32])
            b2d = cst.tile([32, 1], F32)
            nc.sync.dma_start(b2d[:], t["b2c"][32:64])
            b2t = {0: b2u, 1: b2d}
            ax = {}
            for i, nm in enumerate("xyz"):
                a = cst.tile([128, A_ATOM], F32, tag=f"ax{nm}", name=f"ax{nm}")
                nc.sync.dma_start(a[:], t["axyz"][i])
                ax[nm] = a
            nsig, pit = {}, {}
            for s in range(2):
                sg = cst.tile([128, A_ATOM], F32, tag=f"nsig{s}", name=f"nsig{s}")
                nc.sync.dma_start(sg[:], t["sigc"][s])
                nc.vector.tensor_scalar_mul(sg[:], sg[:], -1.0)
                nsig[s] = sg
                p = cst.tile([128, A_ATOM], F32, tag=f"pi{s}", name=f"pi{s}")
                nc.sync.dma_start(p[:], t["pic"][s])
                pit[s] = p
            out_t = cst.tile([128, NSUP], F32, tag="outt", name="outt")

            for sup in range(NSUP):
                orb_u = orbp.tile([32, SUPW * N], F32, tag="orb_u", name="orb_u")
                orb_d = orbp.tile([32, SUPW * N], F32, tag="orb_d", name="orb_d")
                orbst = {0: orb_u, 1: orb_d}
                for blk in range(nblk):
                    r0 = sup * SUPW * N + blk * RCH
                    hT = {}
                    for s in range(2):
                        ps_h = [psH.tile([128, RCH], F32, tag=f"psh{e}", name=f"psh{e}")
                                for e in range(2)]
                        ftile = {}
                        for kc in range(2):
                            ft = ftp.tile([128, RCH], F32, tag=f"ft{s}{kc}", name=f"ft{s}{kc}")
                            nc.sync.dma_start(ft[:], t["fT"][s, kc, :, r0:r0 + RCH])
                            ftile[kc] = ft
                        for ec in range(2):
                            for kc in range(2):
                                nc.tensor.matmul(ps_h[ec][:], w1t[s, kc, ec][:],
                                                 ftile[kc][:],
                                                 start=(kc == 0), stop=(kc == 1))
                            h = hp.tile([128, RCH], F32, tag=f"h{s}{ec}", name=f"h{s}{ec}")
                            nc.scalar.activation(h[:], ps_h[ec][:], AF.Gelu, bias=b1t[s, ec][:])
                            hT[s, ec] = h
                    for s in range(2):
                        ps_o = psO.tile([32, RCH], F32, tag=f"ps_o{s}", name=f"ps_o{s}")
                        for ec in range(2):
                            nc.tensor.matmul(ps_o[:], w2t[s, ec][:], hT[s, ec][:],
                                             start=(ec == 0), stop=(ec == 1))
                        nc.scalar.activation(orbst[s][:, blk * RCH:(blk + 1) * RCH],
                                             ps_o[:], AF.Identity, bias=b2t[s][:])
                Asup = slv.tile([128, 2 * N * N], F32, name="Asup")
                A4 = Asup[:].rearrange("p (s n k) -> p s n k", s=2, n=N)
                for grp in range(4):
                    pt = psT.tile([128, 512], F32, name="pt")
                    for jn in range(8):
                        n = grp * 8 + jn
                        nc.tensor.transpose(pt[:, jn * 64:jn * 64 + 32],
                                            orb_u[:, n::N], ident[0:32, 0:32])
                        nc.tensor.transpose(pt[:, jn * 64 + 32:jn * 64 + 64],
                                            orb_d[:, n::N], ident[0:32, 0:32])
                    src = pt[:].rearrange("p (n s k) -> p n s k", n=8, s=2)
                    dst = A4[:, :, grp * 8:(grp + 1) * 8, :].transpose([0, 2, 1, 3])
                    nc.scalar.activation(dst, src, AF.Copy)
                # envelope
                c_t = {}
                for i, nm in enumerate("xyz"):
                    c = envp.tile([128, 64], F32, tag=f"c{nm}", name=f"c{nm}")
                    nc.sync.dma_start(c[:], t["cxyz"][i, :, sup * 64:(sup + 1) * 64])
                    c_t[nm] = c
                r2 = envp.tile([128, 64 * A_ATOM], F32, tag="r2", name="r2")
                r2v = r2[:].rearrange("p (n a) -> p n a", n=64)
                dbuf = envp.tile([128, 64 * A_ATOM], F32, tag="dbuf", name="dbuf")
                dv = dbuf[:].rearrange("p (n a) -> p n a", n=64)
                for i, nm in enumerate("xyz"):
                    nc.vector.tensor_tensor(
                        dv, c_t[nm][:].unsqueeze(2).broadcast_to([128, 64, A_ATOM]),
                        ax[nm][:].unsqueeze(1).broadcast_to([128, 64, A_ATOM]), op=OP.subtract)
                    if i == 0:
                        nc.vector.tensor_tensor(r2v, dv, dv, op=OP.mult)
                    else:
                        nc.vector.tensor_tensor(dv, dv, dv, op=OP.mult)
                        nc.vector.tensor_tensor(r2v, r2v, dv, op=OP.add)
                nc.scalar.activation(dbuf[:], r2[:], AF.Sqrt)
                for s in range(2):
                    nc.vector.tensor_tensor(
                        r2v[:, s * N:(s + 1) * N, :], dv[:, s * N:(s + 1) * N, :],
                        nsig[s][:].unsqueeze(1).broadcast_to([128, N, A_ATOM]), op=OP.mult)
                nc.scalar.activation(r2[:], r2[:], AF.Exp)
                for s in range(2):
                    nc.vector.tensor_tensor(
                        r2v[:, s * N:(s + 1) * N, :], r2v[:, s * N:(s + 1) * N, :],
                        pit[s][:].unsqueeze(1).broadcast_to([128, N, A_ATOM]), op=OP.mult)
                env = envp.tile([128, 64], F32, tag="env", name="env")
                nc.vector.reduce_sum(env[:], r2v, axis=mybir.AxisListType.X)
                lenv = envp.tile([128, 64], F32, tag="lenv", name="lenv")
                nc.scalar.activation(lenv[:], env[:], AF.Ln)
                lesum = envp.tile([128, 1], F32, tag="lesum", name="lesum")
                nc.vector.reduce_sum(lesum[:], lenv[:], axis=mybir.AxisListType.X)
                # solver
                eliminate(nc, sb, A4, 2, nc.vector)
                dsq = sb.tile([128, 2 * N], F32, tag="dsq", name="dsq")
                diagAP = Asup[:].rearrange("p (s nk) -> p s nk", s=2)[:, :, ::N + 1]
                nc.scalar.activation(dsq[:].rearrange("p (s j) -> p s j", s=2), diagAP, AF.Square)
                lnp = sb.tile([128, 2 * N], F32, tag="lnp", name="lnp")
                nc.scalar.activation(lnp[:], dsq[:], AF.Ln)
                lds = sb.tile([128, 2], F32, tag="lds", name="lds")
                nc.vector.reduce_sum(lds[:], lnp[:].rearrange("p (s j) -> p s j", s=2),
                                     axis=mybir.AxisListType.X)
                tot = sb.tile([128, 1], F32, tag="tot", name="tot")
                nc.vector.tensor_tensor(tot[:], lds[:, 0:1], lds[:, 1:2], op=OP.add)
                nc.vector.tensor_scalar(out_t[:, sup:sup + 1], tot[:], 0.5, 1.0,
                                        op0=OP.mult, op1=OP.bypass)
                nc.vector.tensor_tensor(out_t[:, sup:sup + 1], out_t[:, sup:sup + 1],
                                        lesum[:], op=OP.add)
            nc.sync.dma_start(t["out"][:], out_t[:])
    return t


def pack_core(f_sh, c_sh, common):
    m = dict(common)
    fT = np.empty((2, 2, 128, BL * N), np.float32)
    for s in range(2):
        fs = f_sh[:, s * N:(s + 1) * N, :].reshape(BL * N, D)
        fT[s] = np.ascontiguousarray(fs.T).reshape(2, 128, BL * N)
    m["fT"] = fT
    c = c_sh.reshape(NSUP, SUPW, 64, 3).transpose(3, 1, 0, 2)
    m["cxyz"] = np.ascontiguousarray(c.reshape(3, 128, NSUP * 64))
    return m


def pack_common(atoms, W1s, b1s, W2s, b2s, pis, sigs):
    Q = mix_q()
    W2s = tuple((W2.astype(np.float64) @ Q.astype(np.float64)).astype(np.float32)
                for W2 in W2s)
    b2s = tuple((b2.astype(np.float64) @ Q.astype(np.float64)).astype(np.float32)
                for b2 in b2s)
    m = {}
    W1p = np.empty((2, 2, 2, 128, 128), np.float32)
    for s, W1 in enumerate(W1s):
        for kc in range(2):
            for ec in range(2):
                W1p[s, kc, ec] = W1[kc * 128:(kc + 1) * 128, ec * 128:(ec + 1) * 128]
    m["W1p"] = W1p
    b1p = np.empty((2, 2, 128, 1), np.float32)
    for s, b1 in enumerate(b1s):
        b1p[s] = np.asarray(b1, np.float32).reshape(2, 128, 1)
    m["b1p"] = b1p
    W2p = np.empty((2, 2, 128, 32), np.float32)
    for s, W2 in enumerate(W2s):
        for ec in range(2):
            W2p[s, ec] = W2[ec * 128:(ec + 1) * 128, :]
    m["W2p"] = W2p
    m["b2c"] = np.concatenate([b2s[0], b2s[1]]).reshape(64, 1).astype(np.float32)
    m["axyz"] = np.ascontiguousarray(
        np.broadcast_to(atoms.T[:, None, :], (3, 128, A_ATOM)).astype(np.float32))
    m["sigc"] = np.ascontiguousarray(
        np.broadcast_to(np.stack([sigs[0], sigs[1]])[:, None, :], (2, 128, A_ATOM)).astype(np.float32))
    m["pic"] = np.ascontiguousarray(
        np.broadcast_to(np.stack([pis[0], pis[1]])[:, None, :], (2, 128, A_ATOM)).astype(np.float32))
    return m


_CACHE = {}


def get_compiled():
    if "nc" not in _CACHE:
        nc = bacc.Bacc("TRN2", target_bir_lowering=False, debug=False,
                       num_devices=N_CORES)
        build(nc)
        nc.compile()
        _CACHE["nc"] = nc
    return _CACHE["nc"]


def make_in_maps(features, electron_coords, spins, atom_coords,
                 up_W1, up_b1, up_W2, up_b2, up_pi, up_sigma,
                 down_W1, down_b1, down_W2, down_b2, down_pi, down_sigma):
    f = np.asarray(features, np.float32)
    c = np.asarray(electron_coords, np.float32)
    sp = np.asarray(spins)
    up_idx = np.nonzero(sp[0] > 0)[0][:N]
    dn_idx = np.nonzero(sp[0] < 0)[0][:N]
    idx = np.concatenate([up_idx, dn_idx])
    if not np.array_equal(idx, np.arange(64)):
        f = f[:, idx]
        c = c[:, idx]
    common = pack_common(np.asarray(atom_coords, np.float32),
                         (np.asarray(up_W1, np.float32), np.asarray(down_W1, np.float32)),
                         (np.asarray(up_b1, np.float32), np.asarray(down_b1, np.float32)),
                         (np.asarray(up_W2, np.float32), np.asarray(down_W2, np.float32)),
                         (np.asarray(up_b2, np.float32), np.asarray(down_b2, np.float32)),
                         (np.asarray(up_pi, np.float32), np.asarray(down_pi, np.float32)),
                         (np.asarray(up_sigma, np.float32), np.asarray(down_sigma, np.float32)))
    in_maps = []
    for core in range(N_CORES):
        sl = slice(core * BL, (core + 1) * BL)
        in_maps.append(pack_core(f[sl], c[sl], common))
    return in_maps


def assemble_out(results):
    outs = []
    for core in range(N_CORES):
        o = results[core]["out"]          # [128, NSUP]
        outs.append(np.ascontiguousarray(o.T).reshape(-1))
    return np.concatenate(outs).astype(np.float32)


def kernel(**inputs):
    nc = get_compiled()
    in_maps = make_in_maps(**inputs)
    res = run_bass_kernel_spmd(nc, in_maps, core_ids=list(range(N_CORES)))
    return assemble_out(res.results)
